# revision 1
# baseline (speedup 1.0000x reference)
"""Trainium2 Bass kernel for nn_NeibRoutLayer (capsule-routing GNN message passing).

Strategy (8 NeuronCores, SPMD, no collectives):
  - Nodes are padded to 50176 = 8 cores x 49 tiles x 128 nodes. Each core owns a
    contiguous range of 6272 nodes.
  - Edges are sorted by target node on the CPU (pure index preprocessing) and
    assigned to the core that owns their target.  Within a core, edges are
    grouped per 128-node tile and padded to a multiple of 128 ("chunks").
  - Because every edge lives on the core/tile of its *target*, the segment-sum
    is fully local: no all-reduce is needed at all.
  - Device pipeline per routing iteration, per node tile:
      u_g   = dma_gather(u_table, trg)                      (DMA)
      t     = z * u_g                                       (DVE)
      pav   = avgpool_d(t)          (p = 16*pav)            (DVE pool)
      w     = exp(16 * pav)                                 (ACT)
      r     = 1 / avgpool_c(w)      (r = 8/sum_c exp(p))    (DVE)
      P'    = (iota == trg_local) * r    per 128-edge chunk (DVE tensor_scalar)
      msg   = z * broadcast_d(w)                            (DVE)
      psum += P'^T @ msg                 per chunk          (PE matmul, fp32)
      u_raw = psum * (1/8) + xc                             (DVE STT)
      u_new = l2norm_per_capsule(u_raw)                     (ACT+DVE)
  - z = l2norm(x)[src] is materialized once on device (dma_gather of x rows
    from two zero-row-prefixed half-tables so row indices fit int16, summed,
    then normalized on device), then streamed each iteration.
  - All dma_gathers are issued in <=768-index slices so each SDMA lane stays
    under the 64-descriptor single-packet limit.

kernel(**inputs) takes the FULL inputs and returns the FULL output.
"""

import sys
from contextlib import ExitStack

sys.path.insert(0, "/opt/trn_rl_repo")

import numpy as np

import concourse.bacc as bacc
import concourse.bass as bass
import concourse.tile as tile
from concourse import mybir
from concourse.bass_utils import run_bass_kernel_spmd

# ---------------------------------------------------------------- constants
N_NODES = 50000
D = 128          # feature dim
C = 8            # capsules
DPC = 16         # dims per capsule
NITER = 3
NCORES = 8
T_TILES = 49     # node tiles per core

F32 = mybir.dt.float32
I32 = mybir.dt.int32
I16 = mybir.dt.int16

AF = mybir.ActivationFunctionType
ALU = mybir.AluOpType


def _own():
    return T_TILES * 128


def _npad():
    return NCORES * _own()


# ---------------------------------------------------------------- CPU prep
def _prepare(x, edge_index):
    """Pure index preprocessing: sort edges by target, tile, pad, build
    per-core index arrays and the padded node-feature table."""
    OWN, NPAD = _own(), _npad()
    src = np.asarray(edge_index[0], dtype=np.int64)
    trg = np.asarray(edge_index[1], dtype=np.int64)

    order = np.argsort(trg, kind="stable")
    trg_s = trg[order]
    src_s = src[order]

    n_gtiles = NPAD // 128
    bounds = np.searchsorted(trg_s, np.arange(n_gtiles + 1) * 128)
    tile_cnt = bounds[1:] - bounds[:-1]
    cf = int(np.ceil(max(tile_cnt.max(), 1) / 128))  # chunks per tile
    spt = cf * 128                                   # padded slots per tile

    x_pad = np.ones((NPAD, D), dtype=np.float32)
    x_pad[:N_NODES] = np.asarray(x, dtype=np.float32)

    iota = np.tile(np.arange(128, dtype=np.float32), (128, 1))
    half = NPAD // 2
    xlo = np.vstack([np.zeros((1, D), np.float32), x_pad[:half]])
    xhi = np.vstack([np.zeros((1, D), np.float32), x_pad[half:]])

    in_maps = []
    for c in range(NCORES):
        half = NPAD // 2
        src_slots = np.zeros((T_TILES, spt), dtype=np.int64)
        trgl_slots = np.full((T_TILES, spt), -1.0, dtype=np.float32)
        gidx_slots = np.zeros((T_TILES, spt), dtype=np.int16)
        for j in range(T_TILES):
            g = c * T_TILES + j
            s, e = bounds[g], bounds[g + 1]
            cnt = e - s
            src_slots[j, :cnt] = src_s[s:e]
            trgl_slots[j, :cnt] = (trg_s[s:e] - g * 128).astype(np.float32)
            gidx_slots[j, :cnt] = (trg_s[s:e] - c * OWN).astype(np.int16)

        lo_mask = src_slots < half
        slo_slots = np.where(lo_mask, src_slots + 1, 0).astype(np.int16)
        shi_slots = np.where(lo_mask, 0, src_slots - half + 1).astype(np.int16)

        def wrap16(a):
            # idx i -> partition i%16, col i//16; replicated x8 cores
            w = a.reshape(T_TILES, spt // 16, 16)
            w = np.transpose(w, (2, 0, 1)).reshape(16, T_TILES * (spt // 16))
            return np.ascontiguousarray(np.tile(w, (8, 1)))

        trgl_c = np.ascontiguousarray(trgl_slots.reshape(T_TILES * cf, 128).T)

        in_maps.append({
            "xlo": xlo,
            "xhi": xhi,
            "xown": np.ascontiguousarray(x_pad[c * OWN:(c + 1) * OWN]),
            "slo": wrap16(slo_slots),
            "shi": wrap16(shi_slots),
            "trgl": trgl_c,
            "gidx": wrap16(gidx_slots),
            "iota": iota,
        })
    return cf, in_maps


# ---------------------------------------------------------------- device code
def _build(cf):
    """Build the SPMD Bass program (identical on all 8 cores)."""
    OWN, NPAD = _own(), _npad()
    spt = cf * 128

    nc = bacc.Bacc("TRN2", target_bir_lowering=False, debug=False,
                   num_devices=NCORES)

    half = NPAD // 2
    xlo_in = nc.dram_tensor("xlo", [half + 1, D], F32,
                            kind="ExternalInput").ap()
    xhi_in = nc.dram_tensor("xhi", [half + 1, D], F32,
                            kind="ExternalInput").ap()
    xown_in = nc.dram_tensor("xown", [OWN, D], F32, kind="ExternalInput").ap()
    slo_in = nc.dram_tensor("slo", [128, T_TILES * cf * 8], I16,
                            kind="ExternalInput").ap()
    shi_in = nc.dram_tensor("shi", [128, T_TILES * cf * 8], I16,
                            kind="ExternalInput").ap()
    trgl_in = nc.dram_tensor("trgl", [128, T_TILES * cf], F32,
                             kind="ExternalInput").ap()
    gidx_in = nc.dram_tensor("gidx", [128, T_TILES * cf * 8], I16,
                             kind="ExternalInput").ap()
    iota_in = nc.dram_tensor("iota", [128, 128], F32, kind="ExternalInput").ap()
    u_out = nc.dram_tensor("u_out", [OWN, D], F32, kind="ExternalOutput").ap()

    z_dram = nc.dram_tensor("z_scratch", [T_TILES * spt, D], F32).ap()
    u_dram = nc.dram_tensor("u_table", [OWN, D], F32).ap()

    with tile.TileContext(nc) as tc, ExitStack() as ctx:
        persist = ctx.enter_context(tc.tile_pool(name="persist", bufs=1))
        xc_sb = persist.tile([128, T_TILES * 128], F32, tag="xc")
        ur_sb = persist.tile([128, T_TILES * 128], F32, tag="ur")
        trgl_sb = persist.tile([128, T_TILES * cf], F32, tag="trgl")
        slo_sb = persist.tile([128, T_TILES * cf * 8], I16, tag="slo")
        shi_sb = persist.tile([128, T_TILES * cf * 8], I16, tag="shi")
        gidx_sb = persist.tile([128, T_TILES * cf * 8], I16, tag="gidx")
        iota_sb = persist.tile([128, 128], F32, tag="iota")

        nc.sync.dma_start(out=trgl_sb, in_=trgl_in[:])
        nc.sync.dma_start(out=slo_sb, in_=slo_in[:])
        nc.sync.dma_start(out=shi_sb, in_=shi_in[:])
        nc.sync.dma_start(out=gidx_sb, in_=gidx_in[:])
        nc.sync.dma_start(out=iota_sb, in_=iota_in[:])

        work = ctx.enter_context(tc.tile_pool(name="work", bufs=2))
        small = ctx.enter_context(tc.tile_pool(name="small", bufs=3))
        psum_tp = ctx.enter_context(
            tc.tile_pool(name="psum", bufs=2, space="PSUM"))

        def l2norm_into(dst_ap, src_ap, n_free, sq_tag="sq"):
            """dst = per-capsule l2-normalized src (free width n_free)."""
            ncaps = n_free // DPC
            sq = work.tile([128, n_free], F32, tag=sq_tag)
            nc.scalar.activation(sq, src_ap, AF.Square)
            n2 = small.tile([128, ncaps], F32, tag="n2")
            nc.vector.reduce_sum(
                out=n2, in_=sq.rearrange("p (a b) -> p a b", b=DPC),
                axis=mybir.AxisListType.X)
            nrm = small.tile([128, ncaps], F32, tag="nrm")
            nc.scalar.activation(nrm, n2, AF.Sqrt)
            rn = small.tile([128, ncaps], F32, tag="rn")
            nc.vector.reciprocal(rn, nrm)
            nc.vector.tensor_tensor(
                out=dst_ap.rearrange("p (a b) -> p a b", b=DPC),
                in0=src_ap.rearrange("p (a b) -> p a b", b=DPC),
                in1=rn.to_broadcast([128, ncaps, DPC]),
                op=ALU.mult)


        GMAX = 768  # <= 1008 keeps each SDMA lane under the 64-desc packet cap

        def gather_sliced(dst_ap3, table_ap, idxs_sb, base_col16, n_idxs):
            done = 0
            while done < n_idxs:
                n = min(GMAX, n_idxs - done)
                nc.gpsimd.dma_gather(
                    out_ap=dst_ap3[:, done // 128:(done + n) // 128, :],
                    in_ap=table_ap,
                    idxs_ap=idxs_sb[:, base_col16 + done // 16:
                                    base_col16 + (done + n) // 16],
                    num_idxs=n, num_idxs_reg=n, elem_size=D)
                done += n

        # ---------------- setup: xc for own nodes, u_table = xc ------------
        for t in range(T_TILES):
            xt = work.tile([128, 128], F32, tag="ld128")
            nc.sync.dma_start(out=xt, in_=xown_in[bass.ts(t, 128), :])
            l2norm_into(xc_sb[:, bass.ts(t, 128)], xt, 128)
            nc.sync.dma_start(out=u_dram[bass.ts(t, 128), :],
                              in_=xc_sb[:, bass.ts(t, 128)])

        # ---------------- setup: z = l2norm(x)[src] ------------------------
        for t in range(T_TILES):
            glo = work.tile([128, spt], F32, tag="ug")
            gather_sliced(glo.rearrange("p (a b) -> p a b", b=128),
                          xlo_in[:], slo_sb, t * cf * 8, spt)
            ghi = work.tile([128, spt], F32, tag="msg")
            gather_sliced(ghi.rearrange("p (a b) -> p a b", b=128),
                          xhi_in[:], shi_sb, t * cf * 8, spt)
            xz = work.tile([128, spt], F32, tag="tm")
            nc.vector.tensor_add(out=xz, in0=glo, in1=ghi)
            zt = work.tile([128, spt], F32, tag="z")
            l2norm_into(zt, xz, spt, sq_tag="ld")
            nc.sync.dma_start(
                out=z_dram[bass.ts(t, spt), :].rearrange(
                    "(a p) b -> p a b", p=128),
                in_=zt.rearrange("p (a b) -> p a b", b=128))

        # ---------------- routing iterations ------------------------------
        for it in range(NITER):
            tc.strict_bb_all_engine_barrier()
            # phase A: messages + scatter (ACT does Exp only)
            for t in range(T_TILES):
                ug = work.tile([128, spt], F32, tag="ug")
                gather_sliced(ug.rearrange("p (a b) -> p a b", b=128),
                              u_dram[:], gidx_sb, t * cf * 8, spt)
                zt = work.tile([128, spt], F32, tag="z")
                nc.sync.dma_start(
                    out=zt.rearrange("p (a b) -> p a b", b=128),
                    in_=z_dram[bass.ts(t, spt), :].rearrange(
                        "(a p) b -> p a b", p=128))
                tm = work.tile([128, spt], F32, tag="tm")
                nc.vector.tensor_tensor(out=tm, in0=zt, in1=ug, op=ALU.mult)
                pav = small.tile([128, cf * 8], F32, tag="pav")
                nc.vector.reduce_sum(
                    out=pav, in_=tm.rearrange("p (a b) -> p a b", b=DPC),
                    axis=mybir.AxisListType.X)
                wexp = small.tile([128, cf * 8], F32, tag="wexp")
                nc.scalar.activation(wexp, pav, AF.Exp)
                s8 = small.tile([128, cf], F32, tag="s8")
                nc.vector.reduce_sum(
                    out=s8, in_=wexp.rearrange("p (a b) -> p a b", b=C),
                    axis=mybir.AxisListType.X)
                rr = small.tile([128, cf], F32, tag="rr")
                nc.vector.reciprocal(rr, s8)
                msg = work.tile([128, spt], F32, tag="msg")
                nc.vector.tensor_tensor(
                    out=msg.rearrange("p (a b) -> p a b", b=DPC),
                    in0=zt.rearrange("p (a b) -> p a b", b=DPC),
                    in1=wexp.to_broadcast([128, cf * 8, DPC]),
                    op=ALU.mult)
                ps = psum_tp.tile([128, 128], F32, tag="ps")
                for ch in range(cf):
                    gc = t * cf + ch
                    pm = work.tile([128, 128], F32, tag="pm")
                    nc.vector.tensor_scalar(
                        out=pm,
                        in0=iota_sb,
                        scalar1=trgl_sb[:, gc:gc + 1],
                        scalar2=rr[:, ch:ch + 1],
                        op0=ALU.is_equal,
                        op1=ALU.mult)
                    nc.tensor.matmul(out=ps, lhsT=pm,
                                     rhs=msg[:, bass.ts(ch, 128)],
                                     start=(ch == 0), stop=(ch == cf - 1))
                # u_raw = psum/8 + xc
                nc.vector.scalar_tensor_tensor(
                    out=ur_sb[:, bass.ts(t, 128)],
                    in0=ps,
                    scalar=1.0,
                    in1=xc_sb[:, bass.ts(t, 128)],
                    op0=ALU.mult,
                    op1=ALU.add)
            # phase B: normalize + writeback (ACT does Sqrt only)
            last = it == NITER - 1
            for t in range(T_TILES):
                un = work.tile([128, 128], F32, tag="un")
                l2norm_into(un, ur_sb[:, bass.ts(t, 128)], 128)
                dst = u_out if last else u_dram
                nc.sync.dma_start(out=dst[bass.ts(t, 128), :], in_=un)

    nc.compile()
    return nc


_CACHE = {}


def _get_program(cf):
    if cf not in _CACHE:
        _CACHE[cf] = _build(cf)
    return _CACHE[cf]


def _run(nc, in_maps):
    return run_bass_kernel_spmd(nc, in_maps, list(range(NCORES)))


def kernel(**inputs):
    x = inputs["x"]
    edge_index = inputs["edge_index"]
    cf, in_maps = _prepare(x, edge_index)
    nc = _get_program(cf)
    res = _run(nc, in_maps)
    out = np.concatenate([res.results[c]["u_out"] for c in range(NCORES)],
                         axis=0)
    return np.ascontiguousarray(out[:N_NODES]).astype(np.float32)

